# revision 40
# baseline (speedup 1.0000x reference)
"""Distributed multi-head attention kernel for 8 Trainium2 NeuronCores.

Problem: x[4,2048,1024] -> qkv proj -> 16-head attention (add_zero_attn)
         -> out proj + bias -> [4,2048,1024]

Sharding: 8 cores = 4 batches x 2 query-halves. Each core computes the
full K/V for its batch (KV projection duplicated across the pair, ~4GFLOP,
far cheaper than any 2-rank collective on this fabric) and attention +
output projection for its own 1024 queries. Zero collectives; host
reassembles by concatenation only.

add_zero_attn appends a zero key & value token: the value row is zero so it
only adds +1 to each softmax denominator. We therefore never materialize it;
denominators come from a ones-column appended to V (column 64 of each head's
v tile) and get +1 before the reciprocal.

Schedule notes:
- x is loaded f32 on the sync queue, cast to bf16 on VectorE, and
  transposed on the PE in bf16 (identity matmuls through a psum_s bank,
  ~0.22us per 128x128 block). Chunks 0-5 transpose before attention so
  the S/exp stream starts at ~12us; chunks 6-15 drain inside pair 0.
- Weights load via gpsimd casting-DMA straight to bf16 (no staging).
- All projection work is chopped into small quanta drained inside the
  attention iterations so the PE never idles while ScalarE streams exp.
- Output projection is split into partial (inner chunks c<=5 or 6, plus
  bias) and final (remaining chunks) passes so each token group closes
  as soon as its last oT chunk is normalized. Query-half-0 partials park
  directly in the output and are topped up by an accumulating gpsimd
  DMA during the last pair; query-half-1 partials park in DRAM and are
  added back on VectorE in the tail (8 small groups, plain DMAs).
"""

import sys

sys.path.insert(0, "/opt/trn_rl_repo")

from contextlib import ExitStack

import numpy as np

import concourse.bass as bass
import concourse.tile as tile
from concourse import bacc, mybir
from concourse.masks import make_identity

P = 128
B, N, D = 4, 2048, 1024
H, DH = 16, 64
INNER = H * DH  # 1024
SCALE = DH ** -0.5
NQ = N // 2     # queries per core
NCORES = 8

F32 = mybir.dt.float32
BF16 = mybir.dt.bfloat16
ADD = mybir.AluOpType.add

DC = D // P         # 8 chunks of the model dim
IC = INNER // P     # 8 chunks of the inner dim
TK = N // P         # 16 kpos chunks
NG = H // 2         # 8 head pairs
QB = NQ // 512      # 2 query blocks of 512


def _build_body(ctx: ExitStack, tc, out_ext, x_ext, wqkv_ext, wout_ext, bout_ext):
    nc = tc.nc

    dram_pool = ctx.enter_context(tc.tile_pool(name="dram", bufs=1, space="DRAM"))
    ddram = dram_pool.tile([NG, QB, 2, 512], F32, tag="ddram")
    # parked qb1 output-projection partials (read back in the tail)
    pdram = dram_pool.tile([4, 2, P, 512], F32, tag="pdram")

    consts = ctx.enter_context(tc.tile_pool(name="consts", bufs=1))
    xstage = ctx.enter_context(tc.tile_pool(name="xstage", bufs=3))
    xbfst = ctx.enter_context(tc.tile_pool(name="xbfst", bufs=3))
    wqk_pool = ctx.enter_context(tc.tile_pool(name="wqk", bufs=2))
    wv_pool = ctx.enter_context(tc.tile_pool(name="wv", bufs=1))
    persist = ctx.enter_context(tc.tile_pool(name="persist", bufs=1))
    aw_pool = ctx.enter_context(tc.tile_pool(name="attnw", bufs=3))
    d_pool = ctx.enter_context(tc.tile_pool(name="dsb", bufs=2))
    bcs_pool = ctx.enter_context(tc.tile_pool(name="bcsb", bufs=1))
    outst = ctx.enter_context(tc.tile_pool(name="outst", bufs=4))

    # psum: 2x[128,1024] scores (banks 0-3), 2x[128,512] o-accum (4-5),
    # 2x[128,512] universal for proj/outproj (6-7)
    psum_s = ctx.enter_context(tc.tile_pool(name="psum_s", bufs=2, space="PSUM"))
    psum_o = ctx.enter_context(tc.tile_pool(name="psum_o", bufs=2, space="PSUM"))
    psum_univ = ctx.enter_context(tc.tile_pool(name="psum_univ", bufs=2, space="PSUM"))

    # ---- persistent SBUF arrays ----
    # xT split into 4 independent 512-token slabs (DMA-transpose writes one
    # 128-token column block at a time; consumers read within one slab)
    xT_slabs = [persist.tile([P, DC, 512], BF16, tag=f"xT{s}", name=f"xT_{s}")
                for s in range(4)]

    def xT(c, lo, width):
        s, off = lo // 512, lo % 512
        assert off + width <= 512
        return xT_slabs[s][:, c, off:off + width]

    qT = persist.tile([P, IC, NQ], BF16, tag="qT")
    kT = persist.tile([P, IC, N], BF16, tag="kT")
    v_sb = persist.tile([P, TK, H, DH + 1], BF16, tag="v")   # col DH is ones
    oT = persist.tile([P, IC, NQ], BF16, tag="oT")
    wout = persist.tile([P, IC, D], BF16, tag="wout")

    identity = consts.tile([P, P], BF16, tag="ident")
    make_identity(nc, identity)
    ones_lhsT = consts.tile([1, P], BF16, tag="ones")
    nc.vector.memset(ones_lhsT, 1.0)
    bout_bf = consts.tile([1, D], BF16, tag="bout")
    nc.gpsimd.dma_start(bout_bf, bout_ext)
    d_bf = consts.tile([1, 2, 512], BF16, tag="dbf")

    # ones column of v (written once; v evictions fill the rest)
    for t in range(TK):
        nc.vector.memset(v_sb[:, t, :, DH:DH + 1], 1.0)

    # ---- x pipeline ----
    # chunks 0-7 (critical path): f32 via sync + VectorE cast + PE
    # transpose (bf16 identity matmuls into a psum_s bank, ~0.2us/block).
    # chunks 8-15: bf16 via gpsimd casting-DMA + XBAR DMA transposes
    # (6 on the idle sync queue, 2 drained on scalar).
    xbf_hold = {}

    def chunk_load_sync(r):
        x_f = xstage.tile([P, D], F32, tag="xf", name=f"xf_{r}")
        nc.sync.dma_start(x_f, x_ext[r * P:(r + 1) * P, :])
        xbf_hold[("f", r)] = x_f

    def chunk_load_gp(r):
        x_f = xstage.tile([P, D], F32, tag="xf", name=f"xf_{r}")
        nc.gpsimd.dma_start(x_f, x_ext[r * P:(r + 1) * P, :])
        xbf_hold[("f", r)] = x_f

    def chunk_tp_dma(r, eng):
        s, off = r // 4, (r % 4) * P
        eng.dma_start_transpose(
            xT_slabs[s][:, :, off:off + P], xbf_hold.pop(("b", r)))

    def chunk_cast(r):
        x_b = xbfst.tile([P, D], BF16, tag="xb", name=f"xb_{r}")
        nc.vector.tensor_copy(x_b, xbf_hold.pop(("f", r)))
        xbf_hold[("b", r)] = x_b

    def chunk_tp_pe(r, evict_eng):
        s, off = r // 4, (r % 4) * P
        x_b = xbf_hold.pop(("b", r))
        tp = psum_s.tile([P, D], BF16, tag="st", name=f"tp_{r}")
        for c in range(DC):
            nc.tensor.transpose(tp[:, c * P:(c + 1) * P],
                                x_b[:, c * P:(c + 1) * P], identity)
        dst = xT_slabs[s][:, :, off:off + P]
        src = tp.rearrange("p (c f) -> p c f", c=DC)
        if evict_eng == "scalar":
            nc.scalar.copy(dst, src)
        else:
            nc.vector.tensor_copy(dst, src)

    wqkv_view = wqkv_ext.rearrange("(c p) f -> p c f", p=P)   # [128, DC, 3072]
    wout_view = wout_ext.rearrange("(c p) f -> p c f", p=P)   # [128, IC, 1024]

    def ev_copy(dst, src):
        nc.vector.tensor_copy(dst, src)

    # ---------------- work-quantum builders ----------------
    # Each quantum is a closure doing ~400-900ns of engine work. They are
    # drained a few per attention iteration so the PE never idles while
    # ScalarE streams exp, and ScalarE starts as early as possible.

    def gen_pair_qk(g):
        """Work items computing qT[:,g] and kT[:,g]."""
        items = []
        holder = {}

        def load_w(m, key):
            def run():
                w_b = wqk_pool.tile([P, DC, P], BF16, tag="wqk", name=f"wb_{key}")
                nc.gpsimd.dma_start(w_b, wqkv_view[:, :, m * P:(m + 1) * P])
                holder[key] = w_b
            return run

        # qT column block g: two 512-query halves
        items.append(load_w(g, f"q{g}"))
        for j in range(QB):
            def qa(j=j):
                ps = psum_univ.tile([P, 512], F32, tag="u512", name=f"pq_{g}_{j}")
                holder[f"pq{j}"] = ps
                for c in range(4):
                    nc.tensor.matmul(ps, holder[f"q{g}"][:, c, :],
                                     xT(c, j * 512, 512),
                                     start=(c == 0), stop=False)
            def qb_(j=j):
                ps = holder[f"pq{j}"]
                for c in range(4, DC):
                    nc.tensor.matmul(ps, holder[f"q{g}"][:, c, :],
                                     xT(c, j * 512, 512),
                                     start=False, stop=(c == DC - 1))
            def qe(j=j):
                ev_copy(qT[:, g, j * 512:(j + 1) * 512], holder[f"pq{j}"])
            items += [qa, qb_, qe]

        # kT column block g: four 512-token blocks
        items.append(load_w(8 + g, f"k{g}"))
        for nj in range(4):
            def ka(nj=nj):
                ps = psum_univ.tile([P, 512], F32, tag="u512", name=f"pk_{g}_{nj}")
                holder[f"pk{nj}"] = ps
                for c in range(4):
                    nc.tensor.matmul(ps, holder[f"k{g}"][:, c, :],
                                     xT(c, nj * 512, 512),
                                     start=(c == 0), stop=False)
            def kb(nj=nj):
                ps = holder[f"pk{nj}"]
                for c in range(4, DC):
                    nc.tensor.matmul(ps, holder[f"k{g}"][:, c, :],
                                     xT(c, nj * 512, 512),
                                     start=False, stop=(c == DC - 1))
            def ke(nj=nj):
                ev_copy(kT[:, g, nj * 512:(nj + 1) * 512], holder[f"pk{nj}"])
            items += [ka, kb, ke]
        return items

    def gen_v_half(nh):
        """Work items computing v for heads nh*8 .. nh*8+7 (pairs 4nh..4nh+3).

        v needs kpos on partitions, so x^T chunks are the stationary operand
        and the 8-head weight slab [128, DC, 512] is the moving one (N=512)."""
        items = []
        holder = {}

        def load_wv(blk):
            def run():
                if blk == 0:
                    holder["wvh"] = wv_pool.tile([P, DC, 512], BF16, tag="wvh",
                                                 name=f"wvh_{nh}")
                m = 16 + nh * 4 + blk
                nc.gpsimd.dma_start(
                    holder["wvh"][:, :, blk * P:(blk + 1) * P],
                    wqkv_view[:, :, m * P:(m + 1) * P])
            return run

        for blk in range(4):
            items.append(load_wv(blk))
        for t in range(TK):
            def va(t=t):
                ps = psum_univ.tile([P, 512], F32, tag="u512",
                                    name=f"pv_{t}_{nh}")
                holder[f"pv{t}"] = ps
                for c in range(4):
                    nc.tensor.matmul(ps, xT(c, t * P, P),
                                     holder["wvh"][:, c, :],
                                     start=(c == 0), stop=False)
            def vb(t=t):
                ps = holder[f"pv{t}"]
                for c in range(4, DC):
                    nc.tensor.matmul(ps, xT(c, t * P, P),
                                     holder["wvh"][:, c, :],
                                     start=False, stop=(c == DC - 1))
            def ve(t=t):
                ev_copy(
                    v_sb[:, t, nh * 8:(nh + 1) * 8, 0:DH],
                    holder[f"pv{t}"].rearrange("p (h d) -> p h d", h=8),
                )
            items += [va, vb, ve]
        return items

    def gen_wout_load():
        items = []
        for m in range(DC):
            def run(m=m):
                nc.gpsimd.dma_start(wout[:, :, m * P:(m + 1) * P],
                                    wout_view[:, :, m * P:(m + 1) * P])
            items.append(run)
        return items

    # ---- output projection, split into DRAM partials + accum finals ----
    # All out_ext DMAs ride the gpsimd queue, so the partial write of a
    # group is ordered before its accumulating final by queue FIFO.
    def gen_outproj_partial(qb, c_hi):
        """Partial out-proj (c = 0..c_hi-1 and bias) for query block qb,
        written straight to the output in DRAM."""
        items = []
        holder = {}
        for ti in range(4):
            t = qb * 4 + ti
            for fh in range(2):
                def oa(t=t, fh=fh):
                    ps = psum_univ.tile([P, 512], F32, tag="u512",
                                        name=f"pop_{t}_{fh}")
                    holder[(t, fh)] = ps
                    for c in range(4):
                        nc.tensor.matmul(ps, oT[:, c, t * P:(t + 1) * P],
                                         wout[:, c, fh * 512:(fh + 1) * 512],
                                         start=(c == 0), stop=False)
                def ob(t=t, fh=fh):
                    ps = holder[(t, fh)]
                    for c in range(4, c_hi):
                        nc.tensor.matmul(ps, oT[:, c, t * P:(t + 1) * P],
                                         wout[:, c, fh * 512:(fh + 1) * 512],
                                         start=False, stop=False)
                    nc.tensor.matmul(ps, ones_lhsT,
                                     bout_bf[:, fh * 512:(fh + 1) * 512],
                                     start=False, stop=True)
                def opark(qb=qb, t=t, ti=ti, fh=fh):
                    o_par = outst.tile([P, 512], F32, tag="osb",
                                       name=f"opar_{t}_{fh}")
                    nc.vector.tensor_copy(o_par, holder.pop((t, fh)))
                    if qb == 0:
                        # overwritten later by the accumulating final DMA
                        nc.gpsimd.dma_start(
                            out_ext[t * P:(t + 1) * P,
                                    fh * 512:(fh + 1) * 512], o_par)
                    else:
                        nc.sync.dma_start(pdram[ti, fh], o_par)
                items += [oa, ob, opark]
        return items

    def gen_outproj_final0():
        """Final out-proj for query block 0: c=6,7 matmuls, then a gpsimd
        DMA that accumulates onto the parked partial in the output."""
        items = []
        holder = {}
        for ti in range(4):
            t = ti
            for fh in range(2):
                def fin(t=t, ti=ti, fh=fh):
                    ps = psum_univ.tile([P, 512], F32, tag="u512",
                                        name=f"pof_{t}_{fh}")
                    holder[(ti, fh)] = ps
                    for c in range(6, IC):
                        nc.tensor.matmul(ps, oT[:, c, t * P:(t + 1) * P],
                                         wout[:, c, fh * 512:(fh + 1) * 512],
                                         start=(c == 6), stop=(c == IC - 1))
                def emit(t=t, ti=ti, fh=fh):
                    o_sb = outst.tile([P, 512], F32, tag="osb",
                                      name=f"osb_{t}_{fh}")
                    nc.vector.tensor_copy(o_sb, holder.pop((ti, fh)))
                    nc.gpsimd.dma_start(
                        out_ext[t * P:(t + 1) * P, fh * 512:(fh + 1) * 512],
                        o_sb, accum_op=ADD)
                items += [fin, emit]
        return items

    def gen_outproj_final1():
        """Tail: final out-proj for query block 1: c=7 matmul + VectorE/
        ScalarE add of the pdram partial (prefetched on sync), plain write
        on gpsimd. Avoids slow accumulating DMAs in the serial tail."""
        items = []
        holder = {}
        k = 0
        for ti in range(4):
            t = 4 + ti
            for fh in range(2):
                def pref(ti=ti, fh=fh):
                    # xstage is idle after the prologue; reuse it for the
                    # partial read-back landings
                    ld = xstage.tile([P, 512], F32, tag="xf",
                                     name=f"ld_{ti}_{fh}")
                    nc.sync.dma_start(ld, pdram[ti, fh])
                    holder[("ld", ti, fh)] = ld
                def fin(t=t, ti=ti, fh=fh):
                    ps = psum_univ.tile([P, 512], F32, tag="u512",
                                        name=f"pof_{t}_{fh}")
                    holder[(ti, fh)] = ps
                    nc.tensor.matmul(ps, oT[:, 7, t * P:(t + 1) * P],
                                     wout[:, 7, fh * 512:(fh + 1) * 512],
                                     start=True, stop=True)
                def emit(t=t, ti=ti, fh=fh, k=k):
                    o_sb = outst.tile([P, 512], F32, tag="osb",
                                      name=f"osb_{t}_{fh}")
                    nc.vector.tensor_add(o_sb, holder.pop((ti, fh)),
                                         holder.pop(("ld", ti, fh)))
                    nc.gpsimd.dma_start(
                        out_ext[t * P:(t + 1) * P, fh * 512:(fh + 1) * 512],
                        o_sb)
                items.append((pref, fin, emit))
                k += 1
        # interleave: prefetches run 2 groups ahead
        out = [items[0][0], items[1][0]]
        for k2, (pref, fin, emit) in enumerate(items):
            out.append(fin)
            if k2 + 2 < len(items):
                out.append(items[k2 + 2][0])
            out.append(emit)
        return out

    # ---------------- attention ----------------
    pending = []
    d_store = {}

    def flush_normalize():
        # drain all finished groups: broadcast 1/(rowsum+1) over the 64
        # partitions of each head and normalize oT in place
        while pending:
            gg, gqb = pending.pop(0)
            bc = bcs_pool.tile([P, 512], F32, tag="bcs", name=f"bc_{gg}_{gqb}")
            for h01 in range(2):
                a = ddram[gg, gqb, h01]
                bcast_src = bass.AP(tensor=a.tensor, offset=a.offset,
                                    ap=[[0, 64]] + list(a.ap))
                nc.gpsimd.dma_start(bc[h01 * 64:(h01 + 1) * 64, :], bcast_src)
            nc.vector.reciprocal_approx_fast(bc, bc)
            nc.vector.tensor_mul(
                oT[:, gg, gqb * 512:(gqb + 1) * 512],
                oT[:, gg, gqb * 512:(gqb + 1) * 512],
                bc,
            )

    N_IT = QB * TK

    def spread(items, it_lo, it_hi):
        """Distribute work items evenly across iterations [it_lo, it_hi)."""
        sched = [[] for _ in range(N_IT)]
        span = it_hi - it_lo
        for i in range(span):
            lo = len(items) * i // span
            hi = len(items) * (i + 1) // span
            sched[it_lo + i].extend(items[lo:hi])
        return sched

    def merge(*scheds):
        out = [[] for _ in range(N_IT)]
        for s in scheds:
            for i, lst in enumerate(s):
                out[i].extend(lst)
        return out

    # o-matmuls for the last two kpos chunks of a unit (and its eviction)
    # are deferred into the next unit's first iterations so the next unit's
    # S/exp stream starts without an o-tail bubble on ScalarE
    tail_work = []

    def attention(g, sched):
        """Attention for head pair g; drains sched[it] along the way."""

        def drain(it):
            for w in sched[it]:
                w()

        for qb in range(QB):
            o_ps = [None, None]
            aw_ring = {}
            for kc in range(TK):
                if kc == 8:
                    flush_normalize()
                st = psum_s.tile([P, 1024], F32, tag="st",
                                 name=f"st_{g}_{qb}_{kc}")
                for h01 in range(2):
                    lo = h01 * 64
                    nc.tensor.matmul(
                        st[:, h01 * 512:(h01 + 1) * 512],
                        kT[lo:lo + 64, g, kc * P:(kc + 1) * P],
                        qT[lo:lo + 64, g, qb * 512:(qb + 1) * 512],
                        start=True, stop=True,
                    )
                aw = aw_pool.tile([P, 1024], BF16, tag="aw",
                                  name=f"aw_{g}_{qb}_{kc}")
                nc.scalar.activation(
                    aw, st, mybir.ActivationFunctionType.Exp, scale=SCALE,
                )
                aw_ring[kc] = aw
                drain(qb * TK + kc)
                if kc == 0 and tail_work:
                    tail_work[0]()
                elif kc == 1 and tail_work:
                    tail_work[1]()
                    tail_work.clear()
                if kc >= 2:
                    emit_o(g, qb, kc - 2, o_ps, aw_ring.pop(kc - 2))
            tail_work[:] = [make_tail(g, qb, TK - 2, o_ps, aw_ring.pop(TK - 2),
                                      evict=False),
                            make_tail(g, qb, TK - 1, o_ps, aw_ring.pop(TK - 1),
                                      evict=True)]

    def make_tail(g, qb, kc, o_ps, aw, evict):
        def run():
            emit_o(g, qb, kc, o_ps, aw)
            if evict:
                evict_unit(g, qb, o_ps)
        return run

    def evict_unit(g, qb, o_ps):
        # evict: D rows (+1 for the zero-attn token) and unnormalized oT.
        # d-adds go first so the final flush's reciprocal chain starts
        # early; the last unit is consumed from SBUF, so skip its DMA.
        last = (g == NG - 1 and qb == QB - 1)
        for h01 in range(2):
            d_sb = d_pool.tile([1, 512], F32, tag="dsb",
                               name=f"d_{g}_{qb}_{h01}")
            nc.vector.tensor_scalar_add(d_sb, o_ps[h01][DH:DH + 1, :], 1.0)
            if not last:
                nc.sync.dma_start(ddram[g, qb, h01], d_sb)
            d_store[(g, qb, h01)] = d_sb
        for h01 in range(2):
            nc.vector.tensor_copy(
                oT[h01 * 64:(h01 + 1) * 64, g, qb * 512:(qb + 1) * 512],
                o_ps[h01][0:DH, :],
            )
        pending.append((g, qb))

    def emit_o(g, qb, kc, o_ps, aw):
        for h01 in range(2):
            if kc == 0:
                o_ps[h01] = psum_o.tile([P, 512], F32, tag="ops",
                                        name=f"o_{g}_{qb}_{h01}")
            h = 2 * g + h01
            nc.tensor.matmul(
                o_ps[h01][0:DH + 1, :],
                v_sb[:, kc, h, :],
                aw[:, h01 * 512:(h01 + 1) * 512],
                start=(kc == 0), stop=(kc == TK - 1),
            )

    def final_flush():
        # normalize the last unit without the DRAM broadcast roundtrip:
        # reciprocal on the [1,512] denominators (cast to bf16), then a
        # K=1 matmul broadcast into a psum tile (latency-critical tail)
        gg, gqb = pending.pop(0)
        assert not pending
        bc_ps = psum_univ.tile([P, 512], F32, tag="u512", name="bc_fin")
        for h01 in range(2):
            d_sb = d_store[(gg, gqb, h01)]
            nc.vector.reciprocal_approx_fast(d_sb, d_sb)
            nc.vector.tensor_copy(d_bf[:, h01, :], d_sb)
            nc.tensor.matmul(
                bc_ps[h01 * 64:(h01 + 1) * 64, :],
                ones_lhsT[:, h01 * 64:(h01 + 1) * 64], d_bf[:, h01, :],
                start=True, stop=True,
            )
        nc.vector.tensor_mul(
            oT[:, gg, gqb * 512:(gqb + 1) * 512],
            oT[:, gg, gqb * 512:(gqb + 1) * 512],
            bc_ps,
        )

    # ---------------- main schedule ----------------
    qk0 = gen_pair_qk(0)
    v0 = gen_v_half(0)
    vh1 = gen_v_half(1)     # needed from attention(4); drained in (1)-(3)

    def run(items):
        for w in items:
            w()

    # x ingest: all chunks on sync (f32) + VectorE cast + PE transpose.
    # Chunks 0-5 transpose before attention; 6-15 drain inside pair 0.
    run(qk0[0:1])            # W q0 load (gpsimd)
    run([qk0[7]])            # W k0 load (gpsimd)
    run(v0[0:4])             # wv loads (gpsimd)
    for r in range(3):
        chunk_load_sync(r)

    def chunk_item(r, evict_eng):
        def runit():
            chunk_cast(r)
            if r + 3 < 16:
                chunk_load_sync(r + 3)
            chunk_tp_pe(r, evict_eng)
        return runit

    for r in range(0, 6):
        chunk_item(r, "scalar" if r < 4 else "vector")()
    run(qk0[1:4])            # q j0
    run(qk0[8:11])           # k nj0

    for g in range(NG):
        if g == 0:
            sched = merge(
                spread([chunk_item(r, "scalar" if r % 2 == 0 else "vector")
                        for r in range(6, 16)], 0, 10),
                spread(v0[4:10], 2, 4),      # v t0, t1
                spread(qk0[11:14], 3, 4),    # k nj1 (S kc=4 reads at it 4)
                spread(qk0[14:17], 6, 8),    # k nj2
                spread(qk0[17:20], 10, 12),  # k nj3
                spread(qk0[4:7], 6, 10),     # q j1
                spread(v0[10:28], 4, 10),    # v t2..t7
                spread(v0[28:52], 9, 17),    # v t8..t15
                spread(gen_pair_qk(1), 18, N_IT))
        elif g == 1:
            sched = spread(gen_pair_qk(2) + vh1[:18], 0, N_IT)
        elif g == 2:
            sched = spread(gen_pair_qk(3) + vh1[18:36], 0, N_IT)
        elif g == 3:
            sched = spread(gen_pair_qk(4) + vh1[36:], 0, N_IT)
        elif g == 4:
            sched = spread(gen_pair_qk(5) + gen_wout_load(), 0, N_IT)
        elif g == 5:
            qk7 = gen_pair_qk(7)
            sched = spread(gen_pair_qk(6) + qk7[0:14], 0, N_IT)
        elif g == 6:
            # qb0 oT chunks 0..5 are normalized by end of pair 5
            sched = merge(spread(qk7[14:], 0, N_IT),
                          spread(gen_outproj_partial(0, 6), 4, 30))
        else:
            # qb1 partials incl. c6 ((6,qb1) is flushed at iteration 8);
            # qb0 finals after the (7,qb0) flush at iteration 24
            sched = merge(spread(gen_outproj_partial(1, 7), 9, 23),
                          spread(gen_outproj_final0(), 24, N_IT))
        attention(g, sched)
    for w in tail_work:
        w()
    tail_work.clear()
    final_flush()
    run(gen_outproj_final1())


def build():
    nc = bacc.Bacc("TRN2", target_bir_lowering=False, debug=False,
                   num_devices=NCORES)
    x_ext = nc.dram_tensor("x", [N, D], F32, kind="ExternalInput").ap()
    wqkv_ext = nc.dram_tensor("w_qkv", [D, 3 * INNER], F32, kind="ExternalInput").ap()
    wout_ext = nc.dram_tensor("w_out", [INNER, D], F32, kind="ExternalInput").ap()
    bout_ext = nc.dram_tensor("b_out", [1, D], F32, kind="ExternalInput").ap()
    out_ext = nc.dram_tensor("out", [NQ, D], F32, kind="ExternalOutput").ap()

    with tile.TileContext(nc) as tc:
        with ExitStack() as ctx:
            _build_body(ctx, tc, out_ext, x_ext, wqkv_ext, wout_ext, bout_ext)
    nc.compile()
    return nc


_NC_CACHE = None


def _get_nc():
    global _NC_CACHE
    if _NC_CACHE is None:
        _NC_CACHE = build()
    return _NC_CACHE


def make_in_maps(x, W_qkv, W_out, b_out):
    x = np.ascontiguousarray(np.asarray(x, dtype=np.float32))
    W_qkv = np.ascontiguousarray(np.asarray(W_qkv, dtype=np.float32))
    W_out = np.ascontiguousarray(np.asarray(W_out, dtype=np.float32))
    b_out = np.ascontiguousarray(np.asarray(b_out, dtype=np.float32)).reshape(1, D)
    in_maps = []
    for core in range(NCORES):
        bi, s = core // 2, core % 2
        xb = x[bi]
        if s == 1:  # rotate so this core's queries are rows 0:NQ
            xb = np.concatenate([xb[NQ:], xb[:NQ]], axis=0)
        in_maps.append({
            "x": np.ascontiguousarray(xb),
            "w_qkv": W_qkv,
            "w_out": W_out,
            "b_out": b_out,
        })
    return in_maps


def assemble(outs):
    full = np.empty((B, N, D), np.float32)
    for core in range(NCORES):
        bi, s = core // 2, core % 2
        full[bi, s * NQ:(s + 1) * NQ] = outs[core]
    return full


def kernel(x, W_qkv, W_out, b_out):
    from concourse.bass_utils import run_bass_kernel_spmd

    nc = _get_nc()
    in_maps = make_in_maps(x, W_qkv, W_out, b_out)
    res = run_bass_kernel_spmd(nc, in_maps, core_ids=list(range(NCORES)))
    return assemble([r["out"] for r in res.results])
